# revision 1
# baseline (speedup 1.0000x reference)
"""FBP single-view backprojection kernel for Trainium2 (8 NeuronCores).

Strategy (V1):
  - View-sharded: core c handles views [c*64, (c+1)*64). No communication.
  - Ramp filter as a PE matmul: pf[(b,v), d_out] = sum_k projT[k,(b,v)] * FwR[k,d_out]
    where FwR[k,m] = filt[m-k+511]*w[k] is built on the host from the actual
    filt/w inputs (Toeplitz of the shipped filter times the pre-weight).
  - Backprojection gather: geometry is static, so per-pixel detector indices
    (int16) and interpolation weights W0/W1 = wgt*(1-frac), wgt*frac are
    precomputed on the host. Out-of-range taps are handled by a zero-padded
    detector table (no masks needed).
  - The gather itself runs on GPSIMD ap_gather over a pair-interleaved table
    T2[e] = [P[e], P[e+1]] (d=2), one gather per (pixel, batch); 8 views are
    processed per instruction (one per Q7 core group of 16 partitions).
  - Combine on DVE: out_b = W0*q0 + W1*q1, then strided DMA to the output.
"""
import sys
import numpy as np
from contextlib import ExitStack

sys.path.insert(0, "/opt/trn_rl_repo")

import concourse.bass as bass
import concourse.bacc as bacc
import concourse.mybir as mybir
from concourse.tile import TileContext

# ---------------- problem constants (hardcoded from the task spec) -----------
VIEWS = 512
DETS = 512
H_IMG = 256
W_IMG = 256
D_IMG = 0.006641
D_DET = 0.0072
ANG0 = 0.0
D_ANG = 2.0 * np.pi / VIEWS
S2R = 5.95
D2R = 4.906
VIRDET = D_DET * S2R / (S2R + D2R)

N_CORES = 8
VPC = VIEWS // N_CORES          # views per core = 64
NPIX = H_IMG * W_IMG            # 65536
NGROUPS = VPC // 8              # 8 gather rounds (8 views each)
CHUNK = 1024                    # pixels per ap_gather instruction
NCHUNK = NPIX // CHUNK          # 64
C2 = 16                         # chunks per staging tile (16*8 = 128 rows)
NST = NCHUNK // C2              # 4 stagings per group
NELEMS = 768                    # padded table entries (pairs)
OFF = 128                       # detector index offset into padded table

F32 = mybir.dt.float32
I16 = mybir.dt.int16


# ---------------- host-side static geometry ---------------------------------
def _geometry(v_lo, v_hi):
    """j0 (int16 padded index) and W0/W1 (f32) for views [v_lo, v_hi)."""
    betas = ANG0 + D_ANG * np.arange(VIEWS, dtype=np.float64)[v_lo:v_hi]
    cb = np.cos(betas)[:, None, None]
    sb = np.sin(betas)[:, None, None]
    xs = ((np.arange(W_IMG, dtype=np.float64) - (W_IMG - 1) / 2.0) * D_IMG)[None, None, :]
    ys = (((H_IMG - 1) / 2.0 - np.arange(H_IMG, dtype=np.float64)) * D_IMG)[None, :, None]
    d = S2R - (xs * cb + ys * sb)
    u = S2R * (ys * cb - xs * sb) / d
    wgt = (S2R / d) ** 2
    t = u / VIRDET + (DETS - 1) / 2.0
    i0 = np.floor(t)
    frac = t - i0
    j0 = (i0 + OFF).astype(np.int16)
    assert j0.min() >= 0 and j0.max() < NELEMS - 1, (j0.min(), j0.max())
    W0 = (wgt * (1.0 - frac)).astype(np.float32)
    W1 = (wgt * frac).astype(np.float32)
    nv = v_hi - v_lo
    return (j0.reshape(nv, NPIX), W0.reshape(nv, NPIX), W1.reshape(nv, NPIX))


def _host_static():
    """Per-core IDX and W tables in device layout."""
    IDX = np.empty((N_CORES, NGROUPS, 128, NCHUNK * (CHUNK // 16)), dtype=np.int16)
    W0d = np.empty((N_CORES, NGROUPS, NST, 128, CHUNK), dtype=np.float32)
    W1d = np.empty_like(W0d)
    for c in range(N_CORES):
        j0, W0, W1 = _geometry(c * VPC, (c + 1) * VPC)
        # IDX[g, 16k+p, chunk*(CHUNK//16) + s] = j0[g*8+k, chunk*CHUNK + s*16 + p]
        a = j0.reshape(NGROUPS, 8, NCHUNK, CHUNK // 16, 16)
        IDX[c] = a.transpose(0, 1, 4, 2, 3).reshape(NGROUPS, 128, NCHUNK * (CHUNK // 16))
        # W[g, st, c2*8+k, col] = W[g*8+k, (st*16+c2)*CHUNK + col]
        for W, Wd in ((W0, W0d), (W1, W1d)):
            b = W.reshape(NGROUPS, 8, NST, C2, CHUNK)
            Wd[c] = b.transpose(0, 2, 3, 1, 4).reshape(NGROUPS, NST, 128, CHUNK)
    return IDX, W0d, W1d


_STATIC_CACHE = {}


def host_static():
    if "s" not in _STATIC_CACHE:
        _STATIC_CACHE["s"] = _host_static()
    return _STATIC_CACHE["s"]


# ---------------- bass program ----------------------------------------------
def build_program(ngroups=NGROUPS, nst=NST, debug=False):
    nc = bacc.Bacc("TRN2", target_bir_lowering=False)
    projT = nc.dram_tensor("projT", [128, 4 * 128], F32, kind="ExternalInput")
    FwR = nc.dram_tensor("FwR", [128, 4 * DETS], F32, kind="ExternalInput")
    IDX = nc.dram_tensor("IDX", [NGROUPS, 128, NCHUNK * (CHUNK // 16)], I16, kind="ExternalInput")
    W0d = nc.dram_tensor("W0d", [NGROUPS, NST, 128, C2 * 8 * CHUNK // 128], F32, kind="ExternalInput")
    W1d = nc.dram_tensor("W1d", [NGROUPS, NST, 128, C2 * 8 * CHUNK // 128], F32, kind="ExternalInput")
    out = nc.dram_tensor("out", [2, VPC, NPIX], F32, kind="ExternalOutput")
    if debug:
        dbg_pf = nc.dram_tensor("dbg_pf", [128, DETS], F32, kind="ExternalOutput")
        dbg_t2 = nc.dram_tensor("dbg_t2", [128, 2 * NELEMS], F32, kind="ExternalOutput")
        dbg_g = nc.dram_tensor("dbg_g", [2, 128, 2 * CHUNK], F32, kind="ExternalOutput")
        dbg_s = nc.dram_tensor("dbg_s", [2, 128, 2 * CHUNK], F32, kind="ExternalOutput")

    ex = ExitStack()
    with TileContext(nc) as tc:
        with (
            tc.tile_pool(name="const", bufs=1) as cpool,
            tc.tile_pool(name="psum", bufs=1, space="PSUM") as ppool,
            tc.tile_pool(name="rep", bufs=2) as rpool,
            tc.tile_pool(name="gather", bufs=3) as gpool,
            tc.tile_pool(name="stage", bufs=2) as spool,
            tc.tile_pool(name="work", bufs=3) as wpool,
        ):
            # ---- filter: pf[(b,v), m] via PE ----
            projT_sb = cpool.tile([128, 4 * 128], F32)
            FwR_sb = cpool.tile([128, 4 * DETS], F32)
            nc.sync.dma_start(out=projT_sb[:], in_=projT[:])
            nc.sync.dma_start(out=FwR_sb[:], in_=FwR[:])
            pf_ps = ppool.tile([128, DETS], F32)
            for kc in range(4):
                nc.tensor.matmul(
                    pf_ps[:],
                    lhsT=projT_sb[:, kc * 128:(kc + 1) * 128],
                    rhs=FwR_sb[:, kc * DETS:(kc + 1) * DETS],
                    start=(kc == 0), stop=(kc == 3),
                )
            # ---- padded pair table T2 ----
            tpad = cpool.tile([128, NELEMS + 2], F32)
            nc.vector.memset(tpad[:], 0.0)
            nc.vector.tensor_copy(out=tpad[:, OFF:OFF + DETS], in_=pf_ps[:])
            t2 = cpool.tile([128, 2 * NELEMS], F32)
            nc.vector.tensor_copy(out=t2[:, 0:2 * NELEMS:2], in_=tpad[:, 0:NELEMS])
            nc.vector.tensor_copy(out=t2[:, 1:2 * NELEMS:2], in_=tpad[:, 1:NELEMS + 1])
            # quad table: T4[v, 4e+c] = [P0[e], P0[e+1], P1[e], P1[e+1]]
            t4 = cpool.tile([64, 4 * NELEMS], F32)
            for v in range(VPC):
                for b in range(2):
                    dst = t4[v:v + 1, :].rearrange("p (e c) -> p e c", c=4)[:, :, 2 * b:2 * b + 2]
                    src = t2[b * VPC + v:b * VPC + v + 1, :].rearrange("p (e c) -> p e c", c=2)
                    nc.scalar.dma_start(out=dst, in_=src)
            if debug:
                nc.sync.dma_start(out=dbg_pf[:], in_=tpad[:, OFF:OFF + DETS])
                nc.sync.dma_start(out=dbg_t2[:], in_=t2[:])

            for g in range(ngroups):
                # replicate the 8 views' quad tables across their 16-partition
                # groups (one broadcast DMA: in AP [8 rows][x16 repeat][3072])
                rep = rpool.tile([128, 4 * NELEMS], F32, tag="rep", name="rep")
                src = t4[g * 8:(g + 1) * 8, :]
                src = src.unsqueeze(1).broadcast_to([8, 16, 4 * NELEMS])
                nc.sync.dma_start(out=rep[:], in_=src)
                for st in range(nst):
                    stage = spool.tile([128, 4 * CHUNK], F32, tag="s", name="stage")
                    # all 16 chunks' indices in one DMA
                    idx_t = gpool.tile([128, C2 * CHUNK // 16], I16, tag="idx")
                    w16 = C2 * (CHUNK // 16)
                    nc.sync.dma_start(out=idx_t[:], in_=IDX[g, :, st * w16:(st + 1) * w16])
                    for c2 in range(C2):
                        gt = gpool.tile([128, 4 * CHUNK], F32, tag="g", name="gt")
                        nc.gpsimd.ap_gather(
                            out_ap=gt[:], in_ap=rep[:],
                            idxs_ap=idx_t[:, c2 * (CHUNK // 16):(c2 + 1) * (CHUNK // 16)],
                            channels=128, num_elems=NELEMS, d=4, num_idxs=CHUNK,
                        )
                        # extract the 8 useful rows (one per Q7 group)
                        nc.scalar.dma_start(
                            out=stage[c2 * 8:(c2 + 1) * 8, :],
                            in_=gt[0:128:16, :],
                        )
                        if debug and g == 0 and st == 0 and c2 == 0:
                            nc.sync.dma_start(out=dbg_g[0], in_=gt[:, 0:2 * CHUNK])
                            nc.sync.dma_start(out=dbg_g[1], in_=gt[:, 2 * CHUNK:])
                    if debug and g == 0 and st == 0:
                        nc.sync.dma_start(out=dbg_s[0], in_=stage[:, 0:2 * CHUNK])
                        nc.sync.dma_start(out=dbg_s[1], in_=stage[:, 2 * CHUNK:])
                    w0t = wpool.tile([128, CHUNK], F32, tag="w0")
                    w1t = wpool.tile([128, CHUNK], F32, tag="w1")
                    nc.sync.dma_start(out=w0t[:], in_=W0d[g, st])
                    nc.sync.dma_start(out=w1t[:], in_=W1d[g, st])
                    for b in range(2):
                        t_a = wpool.tile([128, CHUNK], F32, tag="ta")
                        t_o = wpool.tile([128, CHUNK], F32, tag="to")
                        nc.vector.tensor_mul(out=t_a[:], in0=w0t[:], in1=stage[:, 2 * b + 0:4 * CHUNK:4])
                        nc.vector.tensor_mul(out=t_o[:], in0=w1t[:], in1=stage[:, 2 * b + 1:4 * CHUNK:4])
                        nc.vector.tensor_add(out=t_o[:], in0=t_a[:], in1=t_o[:])
                        # rows r = c2*8+k -> out[b, g*8+k, (st*16+c2)*CHUNK + col]
                        dst = out[b, g * 8:(g + 1) * 8, st * C2 * CHUNK:(st * C2 + C2) * CHUNK]
                        dst = dst.rearrange("k (c2 col) -> c2 k col", col=CHUNK)
                        nc.sync.dma_start(out=dst, in_=t_o[:])
    return nc


# ---------------- host runner ------------------------------------------------
def _host_inputs(projection, w, filt):
    """Build per-core input maps."""
    filt = np.asarray(filt, dtype=np.float32).reshape(-1)
    w = np.asarray(w, dtype=np.float32).reshape(-1)
    proj = np.asarray(projection, dtype=np.float32)

    # FwR[k, m] = filt[m - k + 511] * w[k], chunked [128, kc*512 + m]
    k_idx = np.arange(DETS)
    m_idx = np.arange(DETS)
    Fmat = filt[k_idx[:, None] - m_idx[None, :] + DETS - 1] * w[:, None]  # [k, m]
    FwR_dev = Fmat.reshape(4, 128, DETS).transpose(1, 0, 2).reshape(128, 4 * DETS)
    FwR_dev = np.ascontiguousarray(FwR_dev, dtype=np.float32)

    IDX, W0d, W1d = host_static()
    in_maps = []
    for c in range(N_CORES):
        # projT[k, kc*128 + n], n = b*64 + vl, for views c*64+vl
        pv = proj[:, 0, c * VPC:(c + 1) * VPC, :]          # [2, 64, 512]
        pT = pv.reshape(2 * VPC, DETS).T                   # [512(k), 128(n)]
        pT = pT.reshape(4, 128, 128).transpose(1, 0, 2).reshape(128, 4 * 128)
        in_maps.append({
            "projT": np.ascontiguousarray(pT, dtype=np.float32),
            "FwR": FwR_dev,
            "IDX": IDX[c],
            "W0d": W0d[c],
            "W1d": W1d[c],
        })
    return in_maps


_PROGRAM_CACHE = {}


def kernel(projection, w, filt):
    try:
        import profhook  # registers NTFF hook; harmless if absent
    except Exception:
        pass
    from concourse.bass_utils import run_bass_kernel_spmd

    if "nc" not in _PROGRAM_CACHE:
        nc = build_program()
        nc.finalize()
        _PROGRAM_CACHE["nc"] = nc
    nc = _PROGRAM_CACHE["nc"]
    in_maps = _host_inputs(projection, w, filt)
    res = run_bass_kernel_spmd(nc, in_maps, core_ids=list(range(N_CORES)))
    outs = [r["out"] for r in res.results]  # each [2, 64, 65536]
    full = np.concatenate(outs, axis=1)     # [2, 512, 65536]
    return full.reshape(2, VIEWS, H_IMG, W_IMG).astype(np.float32)

